# revision 11
# baseline (speedup 1.0000x reference)
"""Trainium2 Bass kernel for nn_AdaptivePatchEmbedding.

Reference computes, over a [3,1024,1024] image:
  e0: 16x16 patches -> flatten -> @ Wb + b                    (8192 patches)
  e1: 32x32 patches -> bilinear-resize to 16x16 -> @ Wb + b   (4096 patches)
  e2: 64x64 patches -> bilinear-resize to 16x16 -> @ Wb + b   (2048 patches)
plus a ControlNet zero-init MLP branch on e1/e2 that is exactly zero for the
zero mlp weights (host numpy fallback keeps correctness otherwise).

Identities used:
  - 16x16/stride-16 conv == flatten + matmul with Wb = base_w.reshape(D,-1).T
  - bilinear 32->16 (half-pixel) == mean of each 2x2 block
  - bilinear 64->16 == mean of the 2x2 block at rows {4i+1,4i+2} x cols {4j+1,4j+2}

Gather strategy: Trainium indirect DMA moves one contiguous run per
partition per instruction (128 indices max), with ~1.7us fixed cost per
instruction on the GpSimd engine.  To make each WHOLE PATCH one contiguous
run, the host prebuilds 16-row sliding-window tables (channels-last,
pre-averaged for e1/e2):
  T0[z, x, c, r]      = image[c, z+r, x],  r in 0..15        (e0)
  E1RC[p][z, x2, c]   = 2x2 block sum at rows {z,z+1}, cols {2*x2+p, +1}
  T1[p][z, x2, c, r]  = E1RC[p][z+2r, x2, c]                 (e1)
  T2[p][z, x2, c, r]  = E1RC[p][z+4r, x2, c]                 (e2)
One run per e0/e1 patch (2x for e2 + one strided on-chip select), so a
128-patch job is ONE gather instruction -> 14 gathers per core total.
Tables, weights, and the X datapath are bf16 (f32 PSUM accumulation):
full PE stream rate + FWL weight loads + half the gather bytes; the
pre-sums are computed in f32 on the host before the bf16 cast.
The x0.25 resize scale and the +bias epilogue are applied on the host
(cheap numpy on the downloaded result), and the (j,c,r) run ordering is
folded into a host-side row permutation of Wb.

Per core: 14 jobs of 128 patches: gather -> PE-transpose 6 K-tiles
(X [128,768] -> X^T) -> 12 accumulating matmuls vs Wb -> +bias -> DMA out.
Data-parallel over patches across 8 cores; host concatenates outputs.
"""

import os
import sys

for _p in ("/opt/trn_rl_repo", "/root/.axon_site/_ro/trn_rl_repo"):
    if os.path.isdir(_p) and _p not in sys.path:
        sys.path.insert(0, _p)

import numpy as np
import ml_dtypes

BF16 = ml_dtypes.bfloat16

C = 3
H = W = 1024
D = 768
BASE = 16
N0, N1, N2 = 8192, 4096, 2048
NCORES = 8
P0, P1, P2 = N0 // NCORES, N1 // NCORES, N2 // NCORES  # 1024, 512, 256
G0, G1, G2 = P0 // 128, P1 // 128, P2 // 128  # 8, 4, 2 jobs of 128 patches
NJOBS = G0 + G1 + G2

Z0 = H - 15       # 1009: T0 z-range (z + 15 <= 1023)
Z1 = (H - 1) - 30  # 993:  T1 z-range (z + 30 <= 1022)
Z2 = (H - 1) - 60  # 963:  T2 z-range (z + 60 <= 1022)
X2N = 512

_COMPILED = None


def _build_tables(image):
    """Host-side gather tables (sliding-window views + contiguous copies)."""
    imgT = np.ascontiguousarray(image.transpose(1, 2, 0))  # [H, W, C]
    # e0: [z, x, c, r16]
    t0 = np.ascontiguousarray(
        np.lib.stride_tricks.sliding_window_view(imgT, 16, axis=0).astype(BF16)).reshape(-1)
    # row-pair sums [z, x, c], z in 0..1022
    e1r = imgT[:-1] + imgT[1:]
    # + col-pair sums at the two x-phases -> [2, 1023, 512, 3]
    e1rc = np.zeros((2, H - 1, X2N, C), np.float32)
    e1rc[0] = e1r[:, 0::2] + e1r[:, 1::2]
    e1rc[1, :, :511] = e1r[:, 1:-1:2] + e1r[:, 2::2]
    # e1: 16 step-2 rows of E1RC -> [2, Z1, 512, 3, 16]
    t1 = np.ascontiguousarray(
        np.lib.stride_tricks.sliding_window_view(e1rc, 31, axis=1)[..., 0::2].astype(BF16))
    # e2: 16 step-4 rows of E1RC -> [2, Z2, 512, 3, 16]
    t2 = np.ascontiguousarray(
        np.lib.stride_tricks.sliding_window_view(e1rc, 61, axis=1)[..., 0::4].astype(BF16))
    return t0.reshape(-1, 1), t1.reshape(-1, 1), t2.reshape(-1, 1)


def _build_indices(coords0, coords1, coords2):
    """[128, NJOBS] int32 per-patch element offsets (partition = patch-in-job)."""
    idx = np.zeros((128, NJOBS), np.int32)

    c0 = coords0.astype(np.int64).reshape(G0, 128, 2)
    for g in range(G0):
        y, x = c0[g, :, 0], c0[g, :, 1]
        idx[:, g] = ((y * W + x) * (C * BASE)).astype(np.int32)

    c1 = coords1.astype(np.int64).reshape(G1, 128, 2)
    for g in range(G1):
        y, x = c1[g, :, 0], c1[g, :, 1]
        ph = x & 1
        x2 = (x - ph) >> 1
        idx[:, G0 + g] = (((ph * Z1 + y) * X2N + x2) * (C * BASE)).astype(np.int32)

    c2 = coords2.astype(np.int64).reshape(G2, 128, 2)
    for g in range(G2):
        y, x = c2[g, :, 0], c2[g, :, 1]
        ph = (x + 1) & 1
        x2 = (x + 1 - ph) >> 1
        idx[:, G0 + G1 + g] = (((ph * Z2 + (y + 1)) * X2N + x2) * (C * BASE)).astype(np.int32)

    return idx


def _row_perm():
    """Gathered free-dim index (j,c,r) -> logical Wb row c*256 + r*16 + j."""
    fidx = np.arange(D)
    j, rem = np.divmod(fidx, C * BASE)
    c, r = np.divmod(rem, BASE)
    return c * 256 + r * BASE + j


def _build_graph():
    import concourse.bass as bass
    import concourse.mybir as mybir
    from concourse import bacc
    import concourse.tile as tile

    nc = bacc.Bacc("TRN2", target_bir_lowering=False, debug=False)
    f32 = mybir.dt.float32
    bf16 = mybir.dt.bfloat16
    i32 = mybir.dt.int32

    t0_d = nc.dram_tensor("t0", [Z0 * W * C * BASE, 1], bf16, kind="ExternalInput")
    t1_d = nc.dram_tensor("t1", [2 * Z1 * X2N * C * BASE, 1], bf16, kind="ExternalInput")
    t2_d = nc.dram_tensor("t2", [2 * Z2 * X2N * C * BASE, 1], bf16, kind="ExternalInput")
    idx_d = nc.dram_tensor("idx", [128, NJOBS], i32, kind="ExternalInput")
    w_d = nc.dram_tensor("wt", [128, 6 * D], bf16, kind="ExternalInput")
    id_d = nc.dram_tensor("ident", [128, 128], bf16, kind="ExternalInput")
    out_d = nc.dram_tensor("out", [P0 + P1 + P2, D], f32, kind="ExternalOutput")

    NKT = 6

    with tile.TileContext(nc) as tc:
        with (
            tc.tile_pool(name="static", bufs=1) as st,
            tc.tile_pool(name="raw", bufs=4) as raw,
            tc.tile_pool(name="xp", bufs=4) as xp,
            tc.tile_pool(name="psT", bufs=4, space="PSUM") as psT,
            tc.tile_pool(name="psA", bufs=2, space="PSUM") as psA,
            tc.tile_pool(name="outp", bufs=3) as outp,
        ):
            idx_t = st.tile([128, NJOBS], i32, tag="idx")
            nc.sync.dma_start(idx_t[:], idx_d[:])
            id_t = st.tile([128, 128], bf16, tag="id")
            nc.sync.dma_start(id_t[:], id_d[:])
            warm = st.tile([128, 8], bf16, tag="warm")
            nc.scalar.copy(warm[:], id_t[:, 0:8])
            w_t = st.tile([128, 6 * D], bf16, tag="w")

            def load_statics():
                nc.sync.dma_start(w_t[:], w_d[:])

            def gather_job(tbl, job, runw, tag):
                x = raw.tile([128, runw], bf16, tag=tag)
                nc.gpsimd.indirect_dma_start(
                    out=x[:], out_offset=None, in_=tbl[:],
                    in_offset=bass.IndirectOffsetOnAxis(
                        ap=idx_t[:, job:job + 1], axis=0),
                )
                return x

            def embed_job(x_ap, out_row0, jpar):
                xt_sb = xp.tile([128, NKT * 128], bf16, tag="xt")
                for kt in range(NKT):
                    tp = psT.tile([128, 128], bf16, tag="tp")
                    nc.tensor.transpose(tp[:], x_ap[:, kt * 128:(kt + 1) * 128], id_t[:])
                    if kt % 2 == 1:
                        nc.scalar.copy(xt_sb[:, kt * 128:(kt + 1) * 128], tp[:])
                    else:
                        nc.vector.tensor_copy(xt_sb[:, kt * 128:(kt + 1) * 128], tp[:])
                acc0 = psA.tile([128, 512], f32, tag="acc0")
                acc1 = psA.tile([128, 256], f32, tag="acc1")
                for kt in range(NKT):
                    lhs = xt_sb[:, kt * 128:(kt + 1) * 128]
                    nc.tensor.matmul(acc0[:], lhs, w_t[:, kt * D:kt * D + 512],
                                     start=(kt == 0), stop=(kt == NKT - 1))
                    nc.tensor.matmul(acc1[:], lhs, w_t[:, kt * D + 512:(kt + 1) * D],
                                     start=(kt == 0), stop=(kt == NKT - 1))
                o_t = outp.tile([128, D], f32, tag="o")
                nc.scalar.copy(o_t[:, 0:512], acc0[:])
                nc.vector.tensor_copy(o_t[:, 512:768], acc1[:])
                nc.sync.dma_start(out_d[out_row0:out_row0 + 128, :], o_t[:])

            pre = [gather_job(t0_d, g, D, "x0") for g in range(2)]
            load_statics()
            for g in range(G0):
                x = pre[g] if g < 2 else gather_job(t0_d, g, D, "x0")
                embed_job(x[:], g * 128, g % 2)

            for g in range(G1):
                x = gather_job(t1_d, G0 + g, D, "x1")
                embed_job(x[:], P0 + g * 128, g % 2)

            for g in range(G2):
                xr = gather_job(t2_d, G0 + G1 + g, 2 * D, "x2r")
                x = xp.tile([128, D], bf16, tag="x2")
                # select even x2 entries: out[(j,c,r)] = raw[(2j,c,r)]
                xrv = xr[:].rearrange("p (j t) -> p j t", t=C * BASE)
                xv = x[:].rearrange("p (j t) -> p j t", t=C * BASE)
                nc.vector.tensor_copy(xv, xrv[:, 0:32:2, :])
                embed_job(x[:], P0 + P1 + g * 128, g % 2)

    nc.compile()
    return nc


def _get_compiled():
    global _COMPILED
    if _COMPILED is None:
        _COMPILED = _build_graph()
    return _COMPILED


def _mlp_correction(image, coords, g, agg_w, agg_b, mlp_w, mlp_b, base_w, base_b):
    """Host fallback: the zero-init-MLP branch, exact reference math."""
    Wb = base_w.reshape(D, -1).T
    ps = BASE * g
    n = coords.shape[0]
    patches = np.empty((n, C, ps, ps), np.float32)
    for k in range(n):
        y, x = int(coords[k, 0]), int(coords[k, 1])
        patches[k] = image[:, y:y + ps, x:x + ps]
    sub = patches.reshape(n, C, g, BASE, g, BASE).transpose(0, 2, 4, 1, 3, 5)
    sub_e = sub.reshape(n, g, g, C * BASE * BASE) @ Wb + base_b
    agg = np.einsum('nhwd,odhw->no', sub_e, agg_w) + agg_b
    return agg @ mlp_w.T + mlp_b


def build_in_maps(image, coords0, coords1, coords2, base_w, base_b):
    t0, t1, t2 = _build_tables(image)
    Wb = base_w.reshape(D, -1).T  # [768 k, 768 n]
    Wperm = Wb[_row_perm()]
    wtile = Wperm.reshape(6, 128, D).transpose(1, 0, 2).reshape(128, 6 * D)
    wt_np = np.ascontiguousarray(wtile).astype(BF16)
    ident_np = np.eye(128, dtype=np.float32).astype(BF16)

    in_maps = []
    for k in range(NCORES):
        idx = _build_indices(
            coords0[k * P0:(k + 1) * P0],
            coords1[k * P1:(k + 1) * P1],
            coords2[k * P2:(k + 1) * P2],
        )
        in_maps.append(dict(t0=t0, t1=t1, t2=t2, idx=idx,
                            wt=wt_np, ident=ident_np))
    return in_maps


def kernel(image, coords0, coords1, coords2, base_w, base_b,
           agg_w1, agg_b1, agg_w2, agg_b2, mlp_w1, mlp_b1, mlp_w2, mlp_b2):
    from concourse.bass_utils import run_bass_kernel_spmd

    image = np.asarray(image, dtype=np.float32)
    base_w = np.asarray(base_w, dtype=np.float32)
    base_b = np.asarray(base_b, dtype=np.float32)

    nc = _get_compiled()
    in_maps = build_in_maps(image, coords0, coords1, coords2, base_w, base_b)

    res = run_bass_kernel_spmd(nc, in_maps, core_ids=list(range(NCORES)))
    outs = [res.results[k]["out"] for k in range(NCORES)]

    e0 = np.concatenate([o[0:P0] for o in outs], axis=0) + base_b
    e1 = 0.25 * np.concatenate([o[P0:P0 + P1] for o in outs], axis=0) + base_b
    e2 = 0.25 * np.concatenate([o[P0 + P1:] for o in outs], axis=0) + base_b

    if np.any(mlp_w1) or np.any(mlp_b1):
        e1 = e1 + _mlp_correction(image, np.asarray(coords1), 2,
                                  np.asarray(agg_w1, np.float32), np.asarray(agg_b1, np.float32),
                                  np.asarray(mlp_w1, np.float32), np.asarray(mlp_b1, np.float32),
                                  base_w, base_b)
    if np.any(mlp_w2) or np.any(mlp_b2):
        e2 = e2 + _mlp_correction(image, np.asarray(coords2), 4,
                                  np.asarray(agg_w2, np.float32), np.asarray(agg_b2, np.float32),
                                  np.asarray(mlp_w2, np.float32), np.asarray(mlp_b2, np.float32),
                                  base_w, base_b)

    return np.concatenate([e0, e1, e2], axis=0)


# revision 12
# speedup vs baseline: 1.1948x; 1.1948x over previous
"""Trainium2 Bass kernel for nn_AdaptivePatchEmbedding.

Reference computes, over a [3,1024,1024] image:
  e0: 16x16 patches -> flatten -> @ Wb + b                    (8192 patches)
  e1: 32x32 patches -> bilinear-resize to 16x16 -> @ Wb + b   (4096 patches)
  e2: 64x64 patches -> bilinear-resize to 16x16 -> @ Wb + b   (2048 patches)
plus a ControlNet zero-init MLP branch on e1/e2 that is exactly zero for the
zero mlp weights (host numpy fallback keeps correctness otherwise).

Identities used:
  - 16x16/stride-16 conv == flatten + matmul with Wb = base_w.reshape(D,-1).T
  - bilinear 32->16 (half-pixel) == mean of each 2x2 block
  - bilinear 64->16 == mean of the 2x2 block at rows {4i+1,4i+2} x cols {4j+1,4j+2}

Gather strategy: Trainium indirect DMA moves one contiguous run per
partition per instruction (128 indices max), with ~1.7us fixed cost per
instruction on the GpSimd engine.  To make each WHOLE PATCH one contiguous
run, the host prebuilds 16-row sliding-window tables (channels-last,
pre-averaged for e1/e2):
  T0[z, x, c, r]      = image[c, z+r, x],  r in 0..15        (e0)
  E1RC[p][z, x2, c]   = 2x2 block sum at rows {z,z+1}, cols {2*x2+p, +1}
  T1[p][z, x2, c, r]  = E1RC[p][z+2r, x2, c]                 (e1)
  T2[p][z, x2, c, r]  = E1RC[p][z+4r, x2, c]                 (e2)
One run per e0/e1 patch (2x for e2 + one strided on-chip select), so a
128-patch job is ONE gather instruction -> 14 gathers per core total.
Tables, weights, and the X datapath are bf16 (f32 PSUM accumulation):
full PE stream rate + FWL weight loads + half the gather bytes; the
pre-sums are computed in f32 on the host before the bf16 cast.
The x0.25 resize scale and the +bias epilogue are applied on the host
(cheap numpy on the downloaded result), and the (j,c,r) run ordering is
folded into a host-side row permutation of Wb.

Per core: 14 jobs of 128 patches: gather -> PE-transpose 6 K-tiles
(X [128,768] -> X^T) -> 12 accumulating matmuls vs Wb -> +bias -> DMA out.
Data-parallel over patches across 8 cores; host concatenates outputs.
"""

import os
import sys

for _p in ("/opt/trn_rl_repo", "/root/.axon_site/_ro/trn_rl_repo"):
    if os.path.isdir(_p) and _p not in sys.path:
        sys.path.insert(0, _p)

import numpy as np
import ml_dtypes

BF16 = ml_dtypes.bfloat16

C = 3
H = W = 1024
D = 768
BASE = 16
N0, N1, N2 = 8192, 4096, 2048
NCORES = 8
P0, P1, P2 = N0 // NCORES, N1 // NCORES, N2 // NCORES  # 1024, 512, 256
G0, G1, G2 = P0 // 128, P1 // 128, P2 // 128  # 8, 4, 2 jobs of 128 patches
NJOBS = G0 + G1 + G2

Z0 = H - 15       # 1009: T0 z-range (z + 15 <= 1023)
Z1 = (H - 1) - 30  # 993:  T1 z-range (z + 30 <= 1022)
Z2 = (H - 1) - 60  # 963:  T2 z-range (z + 60 <= 1022)
X2N = 512

_COMPILED = None


def _build_tables(image):
    """Host-side gather tables (sliding-window views + contiguous copies)."""
    imgT = np.ascontiguousarray(image.transpose(1, 2, 0))  # [H, W, C]
    # e0: [z, x, c, r16]
    t0 = np.ascontiguousarray(
        np.lib.stride_tricks.sliding_window_view(imgT, 16, axis=0).astype(BF16)).reshape(-1)
    # row-pair sums [z, x, c], z in 0..1022
    e1r = imgT[:-1] + imgT[1:]
    # + col-pair sums at the two x-phases -> [2, 1023, 512, 3]
    e1rc = np.zeros((2, H - 1, X2N, C), np.float32)
    e1rc[0] = e1r[:, 0::2] + e1r[:, 1::2]
    e1rc[1, :, :511] = e1r[:, 1:-1:2] + e1r[:, 2::2]
    # e1: 16 step-2 rows of E1RC -> [2, Z1, 512, 3, 16]
    t1 = np.ascontiguousarray(
        np.lib.stride_tricks.sliding_window_view(e1rc, 31, axis=1)[..., 0::2].astype(BF16))
    # e2: 16 step-4 rows of E1RC -> [2, Z2, 512, 3, 16]
    t2 = np.ascontiguousarray(
        np.lib.stride_tricks.sliding_window_view(e1rc, 61, axis=1)[..., 0::4].astype(BF16))
    return t0.reshape(-1, 1), t1.reshape(-1, 1), t2.reshape(-1, 1)


def _build_indices(coords0, coords1, coords2):
    """[128, NJOBS] int32 per-patch element offsets (partition = patch-in-job)."""
    idx = np.zeros((128, NJOBS), np.int32)

    c0 = coords0.astype(np.int64).reshape(G0, 128, 2)
    for g in range(G0):
        y, x = c0[g, :, 0], c0[g, :, 1]
        idx[:, g] = ((y * W + x) * (C * BASE)).astype(np.int32)

    c1 = coords1.astype(np.int64).reshape(G1, 128, 2)
    for g in range(G1):
        y, x = c1[g, :, 0], c1[g, :, 1]
        ph = x & 1
        x2 = (x - ph) >> 1
        idx[:, G0 + g] = (((ph * Z1 + y) * X2N + x2) * (C * BASE)).astype(np.int32)

    c2 = coords2.astype(np.int64).reshape(G2, 128, 2)
    for g in range(G2):
        y, x = c2[g, :, 0], c2[g, :, 1]
        ph = (x + 1) & 1
        x2 = (x + 1 - ph) >> 1
        idx[:, G0 + G1 + g] = (((ph * Z2 + (y + 1)) * X2N + x2) * (C * BASE)).astype(np.int32)

    return idx


def _row_perm():
    """Gathered free-dim index (j,c,r) -> logical Wb row c*256 + r*16 + j."""
    fidx = np.arange(D)
    j, rem = np.divmod(fidx, C * BASE)
    c, r = np.divmod(rem, BASE)
    return c * 256 + r * BASE + j


def _build_graph():
    import concourse.bass as bass
    import concourse.mybir as mybir
    from concourse import bacc
    import concourse.tile as tile

    nc = bacc.Bacc("TRN2", target_bir_lowering=False, debug=False)
    f32 = mybir.dt.float32
    bf16 = mybir.dt.bfloat16
    i32 = mybir.dt.int32

    t0_d = nc.dram_tensor("t0", [Z0 * W * C * BASE, 1], bf16, kind="ExternalInput")
    t1_d = nc.dram_tensor("t1", [2 * Z1 * X2N * C * BASE, 1], bf16, kind="ExternalInput")
    t2_d = nc.dram_tensor("t2", [2 * Z2 * X2N * C * BASE, 1], bf16, kind="ExternalInput")
    idx_d = nc.dram_tensor("idx", [128, NJOBS], i32, kind="ExternalInput")
    w_d = nc.dram_tensor("wt", [128, 6 * D], bf16, kind="ExternalInput")
    id_d = nc.dram_tensor("ident", [128, 128], bf16, kind="ExternalInput")
    out_d = nc.dram_tensor("out", [P0 + P1 + P2, D], f32, kind="ExternalOutput")

    NKT = 6

    with tile.TileContext(nc) as tc:
        with (
            tc.tile_pool(name="static", bufs=1) as st,
            tc.tile_pool(name="raw", bufs=4) as raw,
            tc.tile_pool(name="xp", bufs=4) as xp,
            tc.tile_pool(name="psT", bufs=4, space="PSUM") as psT,
            tc.tile_pool(name="psA", bufs=2, space="PSUM") as psA,
            tc.tile_pool(name="outp", bufs=3) as outp,
        ):
            idx_t = st.tile([128, NJOBS], i32, tag="idx")
            nc.sync.dma_start(idx_t[:], idx_d[:])
            id_t = st.tile([128, 128], bf16, tag="id")
            nc.sync.dma_start(id_t[:], id_d[:])
            warm = st.tile([128, 8], bf16, tag="warm")
            nc.scalar.copy(warm[:], id_t[:, 0:8])
            w_t = st.tile([128, 6 * D], bf16, tag="w")

            def load_statics():
                nc.sync.dma_start(w_t[:], w_d[:])

            def gather_job(tbl, job, runw, tag):
                x = raw.tile([128, runw], bf16, tag=tag)
                nc.gpsimd.indirect_dma_start(
                    out=x[:], out_offset=None, in_=tbl[:],
                    in_offset=bass.IndirectOffsetOnAxis(
                        ap=idx_t[:, job:job + 1], axis=0),
                )
                return x

            def embed_job(x_ap, out_row0, jpar):
                xt_sb = xp.tile([128, NKT * 128], bf16, tag="xt")
                for kt in range(NKT):
                    tp = psT.tile([128, 128], bf16, tag="tp")
                    nc.tensor.transpose(tp[:], x_ap[:, kt * 128:(kt + 1) * 128], id_t[:])
                    nc.vector.tensor_copy(xt_sb[:, kt * 128:(kt + 1) * 128], tp[:])
                acc0 = psA.tile([128, 512], f32, tag="acc0")
                acc1 = psA.tile([128, 256], f32, tag="acc1")
                for kt in range(NKT):
                    lhs = xt_sb[:, kt * 128:(kt + 1) * 128]
                    nc.tensor.matmul(acc0[:], lhs, w_t[:, kt * D:kt * D + 512],
                                     start=(kt == 0), stop=(kt == NKT - 1))
                    nc.tensor.matmul(acc1[:], lhs, w_t[:, kt * D + 512:(kt + 1) * D],
                                     start=(kt == 0), stop=(kt == NKT - 1))
                o_t = outp.tile([128, D], f32, tag="o")
                nc.scalar.copy(o_t[:, 0:512], acc0[:])
                nc.scalar.copy(o_t[:, 512:768], acc1[:])
                nc.sync.dma_start(out_d[out_row0:out_row0 + 128, :], o_t[:])

            pre = [gather_job(t0_d, g, D, "x0") for g in range(2)]
            load_statics()
            for g in range(G0):
                x = pre[g] if g < 2 else gather_job(t0_d, g, D, "x0")
                embed_job(x[:], g * 128, g % 2)

            for g in range(G1):
                x = gather_job(t1_d, G0 + g, D, "x1")
                embed_job(x[:], P0 + g * 128, g % 2)

            for g in range(G2):
                xr = gather_job(t2_d, G0 + G1 + g, 2 * D, "x2r")
                x = xp.tile([128, D], bf16, tag="x2")
                # select even x2 entries: out[(j,c,r)] = raw[(2j,c,r)]
                xrv = xr[:].rearrange("p (j t) -> p j t", t=C * BASE)
                xv = x[:].rearrange("p (j t) -> p j t", t=C * BASE)
                nc.vector.tensor_copy(xv, xrv[:, 0:32:2, :])
                embed_job(x[:], P0 + P1 + g * 128, g % 2)

    nc.compile()
    return nc


def _get_compiled():
    global _COMPILED
    if _COMPILED is None:
        _COMPILED = _build_graph()
    return _COMPILED


def _mlp_correction(image, coords, g, agg_w, agg_b, mlp_w, mlp_b, base_w, base_b):
    """Host fallback: the zero-init-MLP branch, exact reference math."""
    Wb = base_w.reshape(D, -1).T
    ps = BASE * g
    n = coords.shape[0]
    patches = np.empty((n, C, ps, ps), np.float32)
    for k in range(n):
        y, x = int(coords[k, 0]), int(coords[k, 1])
        patches[k] = image[:, y:y + ps, x:x + ps]
    sub = patches.reshape(n, C, g, BASE, g, BASE).transpose(0, 2, 4, 1, 3, 5)
    sub_e = sub.reshape(n, g, g, C * BASE * BASE) @ Wb + base_b
    agg = np.einsum('nhwd,odhw->no', sub_e, agg_w) + agg_b
    return agg @ mlp_w.T + mlp_b


def build_in_maps(image, coords0, coords1, coords2, base_w, base_b):
    t0, t1, t2 = _build_tables(image)
    Wb = base_w.reshape(D, -1).T  # [768 k, 768 n]
    Wperm = Wb[_row_perm()]
    wtile = Wperm.reshape(6, 128, D).transpose(1, 0, 2).reshape(128, 6 * D)
    wt_np = np.ascontiguousarray(wtile).astype(BF16)
    ident_np = np.eye(128, dtype=np.float32).astype(BF16)

    in_maps = []
    for k in range(NCORES):
        idx = _build_indices(
            coords0[k * P0:(k + 1) * P0],
            coords1[k * P1:(k + 1) * P1],
            coords2[k * P2:(k + 1) * P2],
        )
        in_maps.append(dict(t0=t0, t1=t1, t2=t2, idx=idx,
                            wt=wt_np, ident=ident_np))
    return in_maps


def kernel(image, coords0, coords1, coords2, base_w, base_b,
           agg_w1, agg_b1, agg_w2, agg_b2, mlp_w1, mlp_b1, mlp_w2, mlp_b2):
    from concourse.bass_utils import run_bass_kernel_spmd

    image = np.asarray(image, dtype=np.float32)
    base_w = np.asarray(base_w, dtype=np.float32)
    base_b = np.asarray(base_b, dtype=np.float32)

    nc = _get_compiled()
    in_maps = build_in_maps(image, coords0, coords1, coords2, base_w, base_b)

    res = run_bass_kernel_spmd(nc, in_maps, core_ids=list(range(NCORES)))
    outs = [res.results[k]["out"] for k in range(NCORES)]

    e0 = np.concatenate([o[0:P0] for o in outs], axis=0) + base_b
    e1 = 0.25 * np.concatenate([o[P0:P0 + P1] for o in outs], axis=0) + base_b
    e2 = 0.25 * np.concatenate([o[P0 + P1:] for o in outs], axis=0) + base_b

    if np.any(mlp_w1) or np.any(mlp_b1):
        e1 = e1 + _mlp_correction(image, np.asarray(coords1), 2,
                                  np.asarray(agg_w1, np.float32), np.asarray(agg_b1, np.float32),
                                  np.asarray(mlp_w1, np.float32), np.asarray(mlp_b1, np.float32),
                                  base_w, base_b)
    if np.any(mlp_w2) or np.any(mlp_b2):
        e2 = e2 + _mlp_correction(image, np.asarray(coords2), 4,
                                  np.asarray(agg_w2, np.float32), np.asarray(agg_b2, np.float32),
                                  np.asarray(mlp_w2, np.float32), np.asarray(mlp_b2, np.float32),
                                  base_w, base_b)

    return np.concatenate([e0, e1, e2], axis=0)


# revision 13
# speedup vs baseline: 1.3626x; 1.1405x over previous
"""Trainium2 Bass kernel for nn_AdaptivePatchEmbedding.

Reference computes, over a [3,1024,1024] image:
  e0: 16x16 patches -> flatten -> @ Wb + b                    (8192 patches)
  e1: 32x32 patches -> bilinear-resize to 16x16 -> @ Wb + b   (4096 patches)
  e2: 64x64 patches -> bilinear-resize to 16x16 -> @ Wb + b   (2048 patches)
plus a ControlNet zero-init MLP branch on e1/e2 that is exactly zero for the
zero mlp weights (host numpy fallback keeps correctness otherwise).

Identities used:
  - 16x16/stride-16 conv == flatten + matmul with Wb = base_w.reshape(D,-1).T
  - bilinear 32->16 (half-pixel) == mean of each 2x2 block
  - bilinear 64->16 == mean of the 2x2 block at rows {4i+1,4i+2} x cols {4j+1,4j+2}

Gather strategy: Trainium indirect DMA moves one contiguous run per
partition per instruction (128 indices max), with ~1.7us fixed cost per
instruction on the GpSimd engine.  To make each WHOLE PATCH one contiguous
run, the host prebuilds 16-row sliding-window tables (channels-last,
pre-averaged for e1/e2):
  T0[z, x, c, r]      = image[c, z+r, x],  r in 0..15        (e0)
  E1RC[p][z, x2, c]   = 2x2 block sum at rows {z,z+1}, cols {2*x2+p, +1}
  T1[p][z, x2, c, r]  = E1RC[p][z+2r, x2, c]                 (e1)
  T2[p][z, x2, c, r]  = E1RC[p][z+4r, x2, c]                 (e2)
One run per e0/e1 patch (2x for e2 + one strided on-chip select), so a
128-patch job is ONE gather instruction -> 14 gathers per core total.
Tables, weights, and the X datapath are bf16 (f32 PSUM accumulation):
full PE stream rate + FWL weight loads + half the gather bytes; the
pre-sums are computed in f32 on the host before the bf16 cast.
The x0.25 resize scale and the +bias epilogue are applied on the host
(cheap numpy on the downloaded result), and the (j,c,r) run ordering is
folded into a host-side row permutation of Wb.

Per core: 14 jobs of 128 patches: gather -> PE-transpose 6 K-tiles
(X [128,768] -> X^T) -> 12 accumulating matmuls vs Wb -> +bias -> DMA out.
Data-parallel over patches across 8 cores; host concatenates outputs.
"""

import os
import sys

for _p in ("/opt/trn_rl_repo", "/root/.axon_site/_ro/trn_rl_repo"):
    if os.path.isdir(_p) and _p not in sys.path:
        sys.path.insert(0, _p)

import numpy as np
import ml_dtypes

BF16 = ml_dtypes.bfloat16

C = 3
H = W = 1024
D = 768
BASE = 16
N0, N1, N2 = 8192, 4096, 2048
NCORES = 8
P0, P1, P2 = N0 // NCORES, N1 // NCORES, N2 // NCORES  # 1024, 512, 256
G0, G1, G2 = P0 // 128, P1 // 128, P2 // 128  # 8, 4, 2 jobs of 128 patches
NJOBS = G0 + G1 + G2

Z0 = H - 15       # 1009: T0 z-range (z + 15 <= 1023)
Z1 = (H - 1) - 30  # 993:  T1 z-range (z + 30 <= 1022)
Z2 = (H - 1) - 60  # 963:  T2 z-range (z + 60 <= 1022)
X2N = 512

_COMPILED = None


def _build_tables(image):
    """Host-side gather tables (sliding-window views + contiguous copies)."""
    imgT = np.ascontiguousarray(image.transpose(1, 2, 0))  # [H, W, C]
    # e0: [z, x, c, r16]
    t0 = np.ascontiguousarray(
        np.lib.stride_tricks.sliding_window_view(imgT, 16, axis=0).astype(BF16)).reshape(-1)
    # row-pair sums [z, x, c], z in 0..1022
    e1r = imgT[:-1] + imgT[1:]
    # + col-pair sums at the two x-phases -> [2, 1023, 512, 3]
    e1rc = np.zeros((2, H - 1, X2N, C), np.float32)
    e1rc[0] = e1r[:, 0::2] + e1r[:, 1::2]
    e1rc[1, :, :511] = e1r[:, 1:-1:2] + e1r[:, 2::2]
    # e1: 16 step-2 rows of E1RC -> [2, Z1, 512, 3, 16]
    t1 = np.ascontiguousarray(
        np.lib.stride_tricks.sliding_window_view(e1rc, 31, axis=1)[..., 0::2].astype(BF16))
    # e2: 16 step-4 rows of E1RC -> [2, Z2, 512, 3, 16]
    t2 = np.ascontiguousarray(
        np.lib.stride_tricks.sliding_window_view(e1rc, 61, axis=1)[..., 0::4].astype(BF16))
    return t0.reshape(-1, 1), t1.reshape(-1, 1), t2.reshape(-1, 1)


def _build_indices(coords0, coords1, coords2):
    """[128, NJOBS] int32 per-patch element offsets (partition = patch-in-job)."""
    idx = np.zeros((128, NJOBS), np.int32)

    c0 = coords0.astype(np.int64).reshape(G0, 128, 2)
    for g in range(G0):
        y, x = c0[g, :, 0], c0[g, :, 1]
        idx[:, g] = ((y * W + x) * (C * BASE)).astype(np.int32)

    c1 = coords1.astype(np.int64).reshape(G1, 128, 2)
    for g in range(G1):
        y, x = c1[g, :, 0], c1[g, :, 1]
        ph = x & 1
        x2 = (x - ph) >> 1
        idx[:, G0 + g] = (((ph * Z1 + y) * X2N + x2) * (C * BASE)).astype(np.int32)

    c2 = coords2.astype(np.int64).reshape(G2, 128, 2)
    for g in range(G2):
        y, x = c2[g, :, 0], c2[g, :, 1]
        ph = (x + 1) & 1
        x2 = (x + 1 - ph) >> 1
        idx[:, G0 + G1 + g] = (((ph * Z2 + (y + 1)) * X2N + x2) * (C * BASE)).astype(np.int32)

    return idx


def _row_perm():
    """Gathered free-dim index (j,c,r) -> logical Wb row c*256 + r*16 + j."""
    fidx = np.arange(D)
    j, rem = np.divmod(fidx, C * BASE)
    c, r = np.divmod(rem, BASE)
    return c * 256 + r * BASE + j


def _build_graph():
    import concourse.bass as bass
    import concourse.mybir as mybir
    from concourse import bacc
    import concourse.tile as tile

    nc = bacc.Bacc("TRN2", target_bir_lowering=False, debug=False)
    f32 = mybir.dt.float32
    bf16 = mybir.dt.bfloat16
    i32 = mybir.dt.int32

    t0_d = nc.dram_tensor("t0", [Z0 * W * C * BASE, 1], bf16, kind="ExternalInput")
    t1_d = nc.dram_tensor("t1", [2 * Z1 * X2N * C * BASE, 1], bf16, kind="ExternalInput")
    t2_d = nc.dram_tensor("t2", [2 * Z2 * X2N * C * BASE, 1], bf16, kind="ExternalInput")
    idx_d = nc.dram_tensor("idx", [128, NJOBS], i32, kind="ExternalInput")
    w_d = nc.dram_tensor("wt", [128, 6 * D], bf16, kind="ExternalInput")
    id_d = nc.dram_tensor("ident", [128, 128], bf16, kind="ExternalInput")
    out_d = nc.dram_tensor("out", [P0 + P1 + P2, D], f32, kind="ExternalOutput")

    NKT = 6

    with tile.TileContext(nc) as tc:
        with (
            tc.tile_pool(name="static", bufs=1) as st,
            tc.tile_pool(name="raw", bufs=5) as raw,
            tc.tile_pool(name="xp", bufs=4) as xp,
            tc.tile_pool(name="psT", bufs=4, space="PSUM") as psT,
            tc.tile_pool(name="psA", bufs=2, space="PSUM") as psA,
            tc.tile_pool(name="outp", bufs=6) as outp,
        ):
            idx_t = st.tile([128, NJOBS], i32, tag="idx")
            nc.sync.dma_start(idx_t[:], idx_d[:])
            id_t = st.tile([128, 128], bf16, tag="id")
            nc.sync.dma_start(id_t[:], id_d[:])
            warm = st.tile([128, 8], bf16, tag="warm")
            nc.scalar.copy(warm[:], id_t[:, 0:8])
            w_t = st.tile([128, 6 * D], bf16, tag="w")

            def load_statics():
                nc.sync.dma_start(w_t[:], w_d[:])

            def gather_job(tbl, job, runw, tag):
                x = raw.tile([128, runw], bf16, tag=tag)
                nc.gpsimd.indirect_dma_start(
                    out=x[:], out_offset=None, in_=tbl[:],
                    in_offset=bass.IndirectOffsetOnAxis(
                        ap=idx_t[:, job:job + 1], axis=0),
                )
                return x

            def embed_job(x_ap, out_row0, jpar):
                xt_sb = xp.tile([128, NKT * 128], bf16, tag="xt")
                for kt in range(NKT):
                    tp = psT.tile([128, 128], bf16, tag="tp")
                    nc.tensor.transpose(tp[:], x_ap[:, kt * 128:(kt + 1) * 128], id_t[:])
                    nc.vector.tensor_copy(xt_sb[:, kt * 128:(kt + 1) * 128], tp[:])
                acc0 = psA.tile([128, 512], f32, tag="acc0")
                acc1 = psA.tile([128, 256], f32, tag="acc1")
                for kt in range(NKT):
                    lhs = xt_sb[:, kt * 128:(kt + 1) * 128]
                    nc.tensor.matmul(acc0[:], lhs, w_t[:, kt * D:kt * D + 512],
                                     start=(kt == 0), stop=(kt == NKT - 1))
                    nc.tensor.matmul(acc1[:], lhs, w_t[:, kt * D + 512:(kt + 1) * D],
                                     start=(kt == 0), stop=(kt == NKT - 1))
                o_t = outp.tile([128, D], f32, tag="o")
                nc.scalar.copy(o_t[:, 0:512], acc0[:])
                nc.scalar.copy(o_t[:, 512:768], acc1[:])
                nc.sync.dma_start(out_d[out_row0:out_row0 + 128, :], o_t[:])

            pre = [gather_job(t0_d, g, D, "x0") for g in range(2)]
            load_statics()
            for g in range(G0):
                x = pre[g] if g < 2 else gather_job(t0_d, g, D, "x0")
                embed_job(x[:], g * 128, g % 2)

            for g in range(G1):
                x = gather_job(t1_d, G0 + g, D, "x1")
                embed_job(x[:], P0 + g * 128, g % 2)

            for g in range(G2):
                xr = gather_job(t2_d, G0 + G1 + g, 2 * D, "x2r")
                x = xp.tile([128, D], bf16, tag="x2")
                # select even x2 entries: out[(j,c,r)] = raw[(2j,c,r)]
                xrv = xr[:].rearrange("p (j t) -> p j t", t=C * BASE)
                xv = x[:].rearrange("p (j t) -> p j t", t=C * BASE)
                nc.gpsimd.tensor_copy(xv, xrv[:, 0:32:2, :])
                embed_job(x[:], P0 + P1 + g * 128, g % 2)

    nc.compile()
    return nc


def _get_compiled():
    global _COMPILED
    if _COMPILED is None:
        _COMPILED = _build_graph()
    return _COMPILED


def _mlp_correction(image, coords, g, agg_w, agg_b, mlp_w, mlp_b, base_w, base_b):
    """Host fallback: the zero-init-MLP branch, exact reference math."""
    Wb = base_w.reshape(D, -1).T
    ps = BASE * g
    n = coords.shape[0]
    patches = np.empty((n, C, ps, ps), np.float32)
    for k in range(n):
        y, x = int(coords[k, 0]), int(coords[k, 1])
        patches[k] = image[:, y:y + ps, x:x + ps]
    sub = patches.reshape(n, C, g, BASE, g, BASE).transpose(0, 2, 4, 1, 3, 5)
    sub_e = sub.reshape(n, g, g, C * BASE * BASE) @ Wb + base_b
    agg = np.einsum('nhwd,odhw->no', sub_e, agg_w) + agg_b
    return agg @ mlp_w.T + mlp_b


def build_in_maps(image, coords0, coords1, coords2, base_w, base_b):
    t0, t1, t2 = _build_tables(image)
    Wb = base_w.reshape(D, -1).T  # [768 k, 768 n]
    Wperm = Wb[_row_perm()]
    wtile = Wperm.reshape(6, 128, D).transpose(1, 0, 2).reshape(128, 6 * D)
    wt_np = np.ascontiguousarray(wtile).astype(BF16)
    ident_np = np.eye(128, dtype=np.float32).astype(BF16)

    in_maps = []
    for k in range(NCORES):
        idx = _build_indices(
            coords0[k * P0:(k + 1) * P0],
            coords1[k * P1:(k + 1) * P1],
            coords2[k * P2:(k + 1) * P2],
        )
        in_maps.append(dict(t0=t0, t1=t1, t2=t2, idx=idx,
                            wt=wt_np, ident=ident_np))
    return in_maps


def kernel(image, coords0, coords1, coords2, base_w, base_b,
           agg_w1, agg_b1, agg_w2, agg_b2, mlp_w1, mlp_b1, mlp_w2, mlp_b2):
    from concourse.bass_utils import run_bass_kernel_spmd

    image = np.asarray(image, dtype=np.float32)
    base_w = np.asarray(base_w, dtype=np.float32)
    base_b = np.asarray(base_b, dtype=np.float32)

    nc = _get_compiled()
    in_maps = build_in_maps(image, coords0, coords1, coords2, base_w, base_b)

    res = run_bass_kernel_spmd(nc, in_maps, core_ids=list(range(NCORES)))
    outs = [res.results[k]["out"] for k in range(NCORES)]

    e0 = np.concatenate([o[0:P0] for o in outs], axis=0) + base_b
    e1 = 0.25 * np.concatenate([o[P0:P0 + P1] for o in outs], axis=0) + base_b
    e2 = 0.25 * np.concatenate([o[P0 + P1:] for o in outs], axis=0) + base_b

    if np.any(mlp_w1) or np.any(mlp_b1):
        e1 = e1 + _mlp_correction(image, np.asarray(coords1), 2,
                                  np.asarray(agg_w1, np.float32), np.asarray(agg_b1, np.float32),
                                  np.asarray(mlp_w1, np.float32), np.asarray(mlp_b1, np.float32),
                                  base_w, base_b)
    if np.any(mlp_w2) or np.any(mlp_b2):
        e2 = e2 + _mlp_correction(image, np.asarray(coords2), 4,
                                  np.asarray(agg_w2, np.float32), np.asarray(agg_b2, np.float32),
                                  np.asarray(mlp_w2, np.float32), np.asarray(mlp_b2, np.float32),
                                  base_w, base_b)

    return np.concatenate([e0, e1, e2], axis=0)
